# revision 6
# baseline (speedup 1.0000x reference)
"""Trainium2 Bass kernel for nn_BasicNet (CondConv 3-branch + BN + channel shuffle).

Reference computation:
  x [32, 256, 56, 56] split into 4 channel groups of 64:
    s0 passthrough,
    sq = BN(CondConv3x3(s1)), vr = BN(CondConv3x1(s2)), hz = BN(CondConv1x3(s3))
  out = channel_shuffle(concat([s0, sq, vr, hz]), groups=8)

Sharding: data-parallel over batch (4 samples per core on 8 cores); BN batch
stats (per-channel sum / sum-of-squares) are all-reduced across cores.

Per-core pipeline, per (sample, branch) unit:
  - gpsimd cast-DMA the branch channel slice into a zero-padded f32r SBUF
    image tile (walrus requires fp32r matmul operands to be produced as f32r)
  - DVE reduce -> per-channel spatial sums -> PE matmul vs att_w -> ACT sigmoid
    -> per-sample attention [4]; gpsimd partition_broadcast + DVE
    multiply-accumulate -> per-sample aggregated conv weights (f32r,
    lhsT layout [cin, tap, o])
  - conv as tap-wise fp32r PE matmuls accumulated in PSUM over 7 N-tiles of
    448 (8 output rows); ACT evacuates PSUM->SBUF (upper partition half for
    odd units); DVE bn_stats per PSUM tile
  - bn_aggr -> per-branch sums; AllReduce; BN scale/bias; ACT normalize in
    place; DMA store with the channel shuffle folded into the dest AP.
"""

import sys

if '/opt/trn_rl_repo' not in sys.path:
    sys.path.insert(0, '/opt/trn_rl_repo')

import numpy as np

import concourse.bass as bass
import concourse.bacc as bacc
import concourse.tile as tile
from concourse import mybir
from concourse import bass_utils

F32 = mybir.dt.float32
F32R = mybir.dt.float32r

N_CORES = 8
NS = 4                   # samples per core
H = W = 56
HW = H * W               # 3136
C = 64                   # channels per branch (Cin == O == 64)
KEXP = 4                 # CondConv experts
ROWS_PER_TILE = 8
NT = ROWS_PER_TILE * W   # 448 free elements per matmul tile
N_TILES = H // ROWS_PER_TILE  # 7
M_TOTAL = 32 * HW        # BN stat count
EPS = 1e-5

# branch geometry: (name, pad_h, pad_w, taps as (dy, dx) pairs)
BRANCHES = [
    ('sq', 1, 1, [(dy, dx) for dy in range(3) for dx in range(3)]),
    ('v', 1, 0, [(dy, 0) for dy in range(3)]),
    ('h', 0, 1, [(0, dx) for dx in range(3)]),
]
PAD_SHAPE = {0: (58, 58), 1: (58, 56), 2: (56, 58)}


def _build_nc():
    nc = bacc.Bacc('TRN2', target_bir_lowering=False, debug=False,
                   num_devices=N_CORES)

    x0 = nc.dram_tensor('x0', [NS, C, HW], F32, kind='ExternalInput').ap()
    xp = {}
    for bi, (bn, _, _, _) in enumerate(BRANCHES):
        ph_, pw_ = PAD_SHAPE[bi]
        xp[bi] = nc.dram_tensor(f'xp_{bn}', [NS, C, ph_ * pw_], F32R,
                                kind='ExternalInput').ap()
    w_t = {}
    att_w = {}
    att_b = {}
    gamma = {}
    beta = {}
    for bi, (bn, _, _, taps) in enumerate(BRANCHES):
        w_t[bi] = nc.dram_tensor(f'w_{bn}', [KEXP, C, len(taps), C], F32,
                                 kind='ExternalInput').ap()
        att_w[bi] = nc.dram_tensor(f'aw_{bn}', [C, KEXP], F32,
                                   kind='ExternalInput').ap()
        att_b[bi] = nc.dram_tensor(f'ab_{bn}', [KEXP, 1], F32,
                                   kind='ExternalInput').ap()
        gamma[bi] = nc.dram_tensor(f'g_{bn}', [C, 1], F32,
                                   kind='ExternalInput').ap()
        beta[bi] = nc.dram_tensor(f'b_{bn}', [C, 1], F32,
                                  kind='ExternalInput').ap()
    out = nc.dram_tensor('out', [NS, 4 * C, H, W], F32,
                         kind='ExternalOutput').ap()

    with tile.TileContext(nc) as tc:
        _emit(tc, x0, xp, w_t, att_w, att_b, gamma, beta, out)

    nc.compile()
    return nc


def _emit(tc, x0, xp, w_t, att_w, att_b, gamma, beta, out):
    nc = tc.nc
    from contextlib import ExitStack
    ctx = ExitStack()
    with ctx:
        persist = ctx.enter_context(tc.tile_pool(name='persist', bufs=1))
        aggp = ctx.enter_context(tc.tile_pool(name='aggp', bufs=3))
        smalls = ctx.enter_context(tc.tile_pool(name='smalls', bufs=4))
        psum_conv = ctx.enter_context(
            tc.tile_pool(name='psum_conv', bufs=4, space='PSUM'))
        psum_att = ctx.enter_context(
            tc.tile_pool(name='psum_att', bufs=2, space='PSUM'))
        dram = ctx.enter_context(tc.tile_pool(name='dram', bufs=1, space='DRAM'))

        # ---------- persistent SBUF state ----------
        # padded f32r input image tiles, ping-pong per branch
        in_tiles = {}
        for bi in range(3):
            ph, pw = PAD_SHAPE[bi]
            for pp in range(2):
                t = persist.tile([C, ph * pw], F32R, tag=f'in_{bi}_{pp}',
                                 name=f'in_{bi}_{pp}')
                in_tiles[(bi, pp)] = t

        # expert weights, lhsT layout: [cin, k, tap*64]
        w_sb = {}
        for bi, (bn, _, _, taps) in enumerate(BRANCHES):
            ntap = len(taps)
            t = persist.tile([C, KEXP, ntap * C], F32, tag=f'w_sb_{bi}',
                             name=f'w_sb_{bi}')
            nc.gpsimd.dma_start(out=t, in_=w_t[bi].rearrange('k c x o -> c k (x o)'))
            w_sb[bi] = t

        att_w_sb = {}
        att_b_sb = {}
        for bi in range(3):
            t = persist.tile([C, KEXP], F32, tag=f'aw_sb_{bi}', name=f'aw_sb_{bi}')
            nc.gpsimd.dma_start(out=t, in_=att_w[bi])
            att_w_sb[bi] = t
            t = persist.tile([KEXP, 1], F32, tag=f'ab_sb_{bi}', name=f'ab_sb_{bi}')
            nc.gpsimd.dma_start(out=t, in_=att_b[bi])
            att_b_sb[bi] = t

        g_sb = persist.tile([C, 3], F32, tag='g_sb')
        b_sb = persist.tile([C, 3], F32, tag='b_sb')
        for bi in range(3):
            nc.gpsimd.dma_start(out=g_sb[:, bi:bi + 1], in_=gamma[bi])
            nc.gpsimd.dma_start(out=b_sb[:, bi:bi + 1], in_=beta[bi])

        # conv outputs: 6 tiles, two units each (lower/upper 64 partitions)
        out_tiles = [persist.tile([128, HW], F32, tag=f'out_{i}', name=f'out_{i}')
                     for i in range(6)]

        # per-branch bn_stats: [128(c by half), 2(sample pair), 7(tile), 6]
        bnst = [persist.tile([128, 2, N_TILES, 6], F32, tag=f'bnst_{bi}',
                             name=f'bnst_{bi}')
                for bi in range(3)]

        # ---------- s0 passthrough (channel shuffle folded into AP) ----------
        ov = out.rearrange('n (c2 g) h w -> n g c2 (h w)', g=8)
        nc.sync.dma_start(out=ov[:, 0], in_=x0[:, 0:32])
        nc.sync.dma_start(out=ov[:, 1], in_=x0[:, 32:64])

        # ---------- per (sample, branch) units ----------
        for s in range(NS):
            for bi, (bn, pad_h, pad_w, taps) in enumerate(BRANCHES):
                u = s * 3 + bi
                half = u % 2
                p0 = 64 * half
                otile = out_tiles[u // 2]
                ntap = len(taps)
                ph, pw = PAD_SHAPE[bi]

                # input DMA: host pre-padded f32r image, contiguous HWDGE copy
                it = in_tiles[(bi, s % 2)]
                it3 = it.rearrange('c (r q) -> c r q', q=pw)
                nc.sync.dma_start(out=it, in_=xp[bi][s])

                # attention: pooled sums -> sigmoid(att_w @ mean + b)
                pooled = smalls.tile([C, 1], F32, tag='pooled')
                nc.vector.tensor_reduce(out=pooled, in_=it.bitcast(F32),
                                        axis=mybir.AxisListType.X,
                                        op=mybir.AluOpType.add)
                att_ps = psum_att.tile([KEXP, 1], F32, tag='att_ps')
                nc.tensor.matmul(att_ps, lhsT=att_w_sb[bi], rhs=pooled,
                                 start=True, stop=True)
                att_s = smalls.tile([KEXP, 1], F32, tag='att_s')
                nc.scalar.activation(out=att_s, in_=att_ps,
                                     func=mybir.ActivationFunctionType.Sigmoid,
                                     bias=att_b_sb[bi])
                att_f = smalls.tile([1, KEXP], F32, tag='att_f')
                nc.gpsimd.dma_start(out=att_f, in_=att_s)
                att_bc = smalls.tile([C, KEXP], F32, tag='att_bc')
                nc.gpsimd.partition_broadcast(att_bc, att_f)

                # aggregate per-sample conv weights: agg = sum_k att[k] * w[k]
                agg = aggp.tile([C, ntap * C], F32, tag='agg')
                nc.vector.tensor_scalar_mul(out=agg, in0=w_sb[bi][:, 0],
                                            scalar1=att_bc[:, 0:1])
                for k in range(1, KEXP - 1):
                    nc.vector.scalar_tensor_tensor(
                        out=agg, in0=w_sb[bi][:, k], scalar=att_bc[:, k:k + 1],
                        in1=agg, op0=mybir.AluOpType.mult, op1=mybir.AluOpType.add)
                agg_r = aggp.tile([C, ntap * C], F32R, tag='agg_r')
                nc.vector.scalar_tensor_tensor(
                    out=agg_r, in0=w_sb[bi][:, KEXP - 1],
                    scalar=att_bc[:, KEXP - 1:KEXP], in1=agg,
                    op0=mybir.AluOpType.mult, op1=mybir.AluOpType.add)

                # conv: per N-tile, accumulate taps in PSUM partitions 0:64
                for t in range(N_TILES):
                    pt = psum_conv.tile([64, NT], F32, tag='pt')
                    for ti, (dy, dx) in enumerate(taps):
                        r0 = ROWS_PER_TILE * t + dy
                        rhs = it3[:, r0:r0 + ROWS_PER_TILE, dx:dx + W]
                        nc.tensor.matmul(
                            pt, lhsT=agg_r[:, ti * C:(ti + 1) * C], rhs=rhs,
                            start=(ti == 0), stop=(ti == ntap - 1))
                    # evacuate (cross-partition for odd units) + stats
                    nc.scalar.activation(
                        out=otile[p0:p0 + 64, t * NT:(t + 1) * NT], in_=pt,
                        func=mybir.ActivationFunctionType.Copy)
                    nc.vector.bn_stats(out=bnst[bi][p0:p0 + 64, s // 2, t, :],
                                       in_=pt)

        # ---------- BN stats all-reduce ----------
        CNT = 2 * N_TILES * NT           # elements behind each bn_aggr output
        cc_in = dram.tile([3, 2, 2, C], F32)   # (branch, stat, half, channel)
        cc_out = dram.tile([3, 2, 2, C], F32)
        for bi in range(3):
            red_mv = smalls.tile([128, 2], F32, tag='red_mv')
            nc.vector.bn_aggr(out=red_mv, in_=bnst[bi])
            red2 = smalls.tile([128, 2], F32, tag='red2')
            # sum = mean*CNT ; sumsq = (var + mean^2)*CNT
            nc.vector.tensor_scalar_mul(out=red2[:, 0:1], in0=red_mv[:, 0:1],
                                        scalar1=float(CNT))
            tmp = smalls.tile([128, 1], F32, tag='tmp_red')
            nc.vector.tensor_tensor(out=tmp, in0=red_mv[:, 0:1],
                                    in1=red_mv[:, 0:1], op=mybir.AluOpType.mult)
            nc.vector.tensor_tensor(out=tmp, in0=tmp, in1=red_mv[:, 1:2],
                                    op=mybir.AluOpType.add)
            nc.vector.tensor_scalar_mul(out=red2[:, 1:2], in0=tmp,
                                        scalar1=float(CNT))
            nc.gpsimd.dma_start(
                out=cc_in[bi].rearrange('stat half c -> half c stat'), in_=red2)
        nc.gpsimd.collective_compute(
            'AllReduce', mybir.AluOpType.add,
            replica_groups=[list(range(N_CORES))],
            ins=[cc_in.opt()], outs=[cc_out.opt()])

        gsums = persist.tile([C, 3, 2, 2], F32, tag='gsums')
        nc.gpsimd.dma_start(out=gsums,
                            in_=cc_out.rearrange('b stat half c -> c b stat half'))
        gs = persist.tile([C, 3, 2], F32, tag='gs')
        nc.vector.tensor_reduce(out=gs, in_=gsums, axis=mybir.AxisListType.X,
                                op=mybir.AluOpType.add)
        # mean / E[x^2] -> scale/bias
        mv = persist.tile([C, 3, 2], F32, tag='mv')
        nc.vector.tensor_scalar_mul(out=mv, in0=gs, scalar1=1.0 / M_TOTAL)
        var = persist.tile([C, 3], F32, tag='var')
        nc.vector.tensor_tensor(out=var, in0=mv[:, :, 0], in1=mv[:, :, 0],
                                op=mybir.AluOpType.mult)
        nc.vector.tensor_tensor(out=var, in0=mv[:, :, 1], in1=var,
                                op=mybir.AluOpType.subtract)
        sd = persist.tile([C, 3], F32, tag='sd')
        epst = persist.tile([C, 1], F32, tag='epst')
        nc.vector.memset(epst, EPS)
        nc.scalar.activation(out=sd, in_=var,
                             func=mybir.ActivationFunctionType.Sqrt, bias=epst)
        nc.vector.reciprocal(out=sd, in_=sd)
        scale2 = persist.tile([128, 3], F32, tag='scale2')
        bias2 = persist.tile([128, 3], F32, tag='bias2')
        nc.vector.tensor_tensor(out=scale2[0:64], in0=g_sb, in1=sd,
                                op=mybir.AluOpType.mult)
        tmpb = persist.tile([C, 3], F32, tag='tmpb')
        nc.vector.tensor_tensor(out=tmpb, in0=mv[:, :, 0], in1=scale2[0:64],
                                op=mybir.AluOpType.mult)
        nc.vector.tensor_tensor(out=bias2[0:64], in0=b_sb, in1=tmpb,
                                op=mybir.AluOpType.subtract)
        nc.gpsimd.dma_start(out=scale2[64:128], in_=scale2[0:64])
        nc.gpsimd.dma_start(out=bias2[64:128], in_=bias2[0:64])

        # ---------- normalize + store (shuffle folded into dest AP) ----------
        for s in range(NS):
            for bi in range(3):
                u = s * 3 + bi
                half = u % 2
                p0 = 64 * half
                otile = out_tiles[u // 2]
                oh = otile[p0:p0 + 64, :]
                nc.scalar.activation(out=oh, in_=oh,
                                     func=mybir.ActivationFunctionType.Identity,
                                     bias=bias2[p0:p0 + 64, bi:bi + 1],
                                     scale=scale2[p0:p0 + 64, bi:bi + 1])
                g1 = 2 * (bi + 1)
                nc.sync.dma_start(out=ov[s, g1], in_=otile[p0:p0 + 32, :])
                nc.sync.dma_start(out=ov[s, g1 + 1], in_=otile[p0 + 32:p0 + 64, :])


_NC_CACHE = None


def _get_nc():
    global _NC_CACHE
    if _NC_CACHE is None:
        _NC_CACHE = _build_nc()
    return _NC_CACHE


def _prep_in_maps(inputs):
    x = np.ascontiguousarray(inputs['x'], dtype=np.float32)
    n_total = x.shape[0]
    # host-side zero padding per branch (pad folded into the shipped tensor)
    pads = [(1, 1), (1, 0), (0, 1)]
    xpad = []
    for bi in range(3):
        ph_, pw_ = pads[bi]
        sl = x[:, C * (bi + 1):C * (bi + 2)]
        p = np.zeros((n_total, C, H + 2 * ph_, W + 2 * pw_), np.float32)
        p[:, :, ph_:ph_ + H, pw_:pw_ + W] = sl
        xpad.append(np.ascontiguousarray(p.reshape(n_total, C, -1)))
    x0_full = np.ascontiguousarray(x[:, 0:C].reshape(n_total, C, HW))
    shared = {}
    for bn, wk, awk, abk, gk, bk in [
            ('sq', 'w_sq', 'att_w_sq', 'att_b_sq', 'g_sq', 'b_sq'),
            ('v', 'w_v', 'att_w_v', 'att_b_v', 'g_v', 'b_v'),
            ('h', 'w_h', 'att_w_h', 'att_b_h', 'g_h', 'b_h')]:
        w = np.asarray(inputs[wk], dtype=np.float32)          # [K, O, Cin, kh, kw]
        k, o, cin, kh, kw = w.shape
        shared[f'w_{bn}'] = np.ascontiguousarray(
            w.transpose(0, 2, 3, 4, 1).reshape(k, cin, kh * kw, o))
        shared[f'aw_{bn}'] = np.ascontiguousarray(
            np.asarray(inputs[awk], dtype=np.float32).T / float(HW))
        shared[f'ab_{bn}'] = np.ascontiguousarray(
            inputs[abk], dtype=np.float32).reshape(KEXP, 1)
        shared[f'g_{bn}'] = np.ascontiguousarray(
            inputs[gk], dtype=np.float32).reshape(C, 1)
        shared[f'b_{bn}'] = np.ascontiguousarray(
            inputs[bk], dtype=np.float32).reshape(C, 1)

    in_maps = []
    for ci in range(N_CORES):
        m = dict(shared)
        sl = slice(ci * NS, (ci + 1) * NS)
        m['x0'] = x0_full[sl]
        for bi, (bn, _, _, _) in enumerate(BRANCHES):
            m[f'xp_{bn}'] = xpad[bi][sl]
        in_maps.append(m)
    return in_maps


def run_raw(inputs, trace=False, **kwargs):
    """Build+run; returns (full_output, BassKernelResults)."""
    nc = _get_nc()
    in_maps = _prep_in_maps(inputs)
    res = bass_utils.run_bass_kernel_spmd(
        nc, in_maps, core_ids=list(range(N_CORES)), trace=trace, **kwargs)
    full = np.concatenate([res.results[i]['out'] for i in range(N_CORES)], axis=0)
    return full, res


def kernel(**inputs):
    full, _ = run_raw(inputs)
    return full
